# revision 1
# baseline (speedup 1.0000x reference)
"""Trainium2 Bass kernel for nn_Cluster_kmeans_pp (VQ codebook EMA update).

Computation (matches the CPU/XLA reference semantics exactly):
  1. z[b]     = argmin_k ||y_b - m_k||^2           (distance GEMM + argmin)
  2. winner_k = max{b : z[b] = k}                  (scatter last-write-wins)
  3. new_m[k] = 0.01*m[k] + 0.99*y[winner_k]       (assigned k only)
     new_sd[k]= (new_m[k]-y[winner_k])^2*0.01 + 0.99*sd[k]
  4. out = concat([new_m, new_sd], axis=0)

Distribution over 8 NeuronCores:
  - Phase 1 data-parallel over batch: core i computes scores for batches
    [512*i, 512*(i+1)) x all 1024 clusters via a split-bf16 (hi/lo) GEMM
    (3 bf16 matmuls ~ fp32 precision at bf16 throughput), finds each row's
    argmax of (y.m - 0.5||m||^2) via row-max + equality mask, and encodes
    the per-cluster winning batch (b+1, 0 if none) with PE-transposes +
    max reductions across the batch partitions.
  - Phase 2: 4KB AllReduce(max) of the [1024] winner encodings.
  - Phase 3 cluster-parallel: core i owns clusters [128*i, 128*(i+1));
    indirect-DMA gathers y[winner] rows and applies the EMA updates.

Inputs are host-packed into a handful of large contiguous DMAs (the SWDGE
issue path costs ~1us per dma_start, so hundreds of small DMAs dominate
otherwise).
"""

import sys

if "/opt/trn_rl_repo" not in sys.path:
    sys.path.insert(0, "/opt/trn_rl_repo")

import numpy as np
import ml_dtypes

import concourse.bass as bass
import concourse.mybir as mybir
import concourse.tile as tile
from concourse import bacc
from concourse.bass_utils import run_bass_kernel_spmd

BF16 = ml_dtypes.bfloat16
N_CORES = 8
P = 128
B, C, L = 4096, 64, 64
CL = C * L              # 4096 contraction dim
N_CLUST = 1024
B_SH = B // N_CORES     # 512 batches per core
K_SH = N_CLUST // N_CORES  # 128 clusters per core
NCI = CL // P           # 32 contraction chunks (ci)
NJ = B_SH // P          # 4 batch subtiles per core
KCHUNKS = N_CLUST // P  # 8 cluster chunks
YG = 4                  # yT resident groups (8 ci each)
MG = 8                  # mT stream groups per half (4 ci each)
MG_CI = NCI // MG       # 4 ci per mT group

_CACHE = {}


def _build():
    nc = bacc.Bacc("TRN2", target_bir_lowering=False, debug=False, num_devices=N_CORES)
    f32 = mybir.dt.float32
    bf16 = mybir.dt.bfloat16

    # packed phase-1 operands (see _prep_inputs for layouts)
    yT_pack = nc.declare_dram_parameter("yT_pack", [YG * P, (NCI // YG) * 2 * B_SH],
                                        bf16, isOutput=False)
    mT_pack = nc.declare_dram_parameter("mT_pack", [2 * MG * P, MG_CI * 2 * 512],
                                        bf16, isOutput=False)
    msqneg = nc.declare_dram_parameter("msqneg", [P, N_CLUST], f32, isOutput=False)
    m_nat = nc.declare_dram_parameter("m_nat", [K_SH, CL], f32, isOutput=False)
    sd_nat = nc.declare_dram_parameter("sd_nat", [K_SH, CL], f32, isOutput=False)
    # y split into column halves so the gathers/EMA chains pipeline
    # (finer splits lose: ~0.4us fixed overhead per DVE op)
    NQ = 2
    y_q = [nc.declare_dram_parameter(f"y_q{q}", [B, CL // NQ], f32, isOutput=False)
           for q in range(NQ)]
    bglob = nc.declare_dram_parameter("bglob", [P, NJ], f32, isOutput=False)
    ident_in = nc.declare_dram_parameter("ident", [P, P], f32, isOutput=False)
    out = nc.declare_dram_parameter("out", [2 * K_SH, CL], f32, isOutput=True)

    # winner exchange: transposed [kc, p] layout so ReduceScatter(max) hands
    # core i exactly its own cluster chunk
    cc_in = nc.dram_tensor("cc_in", [KCHUNKS, P], f32)
    rs_out = nc.dram_tensor("rs_out", [1, P], f32)
    core_ids = list(range(N_CORES))

    AO = mybir.AluOpType
    AX = mybir.AxisListType

    with tile.TileContext(nc) as tc:
        with tc.tile_pool(name="const", bufs=1) as cpool, \
             tc.tile_pool(name="p3big", bufs=1) as sb3:
            ident = cpool.tile([P, P], f32)
            nc.gpsimd.dma_start(out=ident[:], in_=ident_in[:])
            msq_t = cpool.tile([P, N_CLUST], f32)
            nc.gpsimd.dma_start(out=msq_t[:], in_=msqneg[:])
            bg_t = cpool.tile([P, NJ], f32)
            nc.gpsimd.dma_start(out=bg_t[:], in_=bglob[:])
            # phase-3 data; DMAs issued mid-stream (below) so they don't
            # steal head bandwidth from the first matmul operands
            m_sb = sb3.tile([K_SH, CL], f32)
            sd_sb = sb3.tile([K_SH, CL], f32)

            # ---------------- Phase 1: score GEMM ----------------
            with tc.tile_pool(name="p1sbuf", bufs=1) as sb1, \
                 tc.tile_pool(name="mstream", bufs=5) as mpool, \
                 tc.tile_pool(name="psum8", bufs=8, space="PSUM") as ps8:

                # tiny first-chunk tiles so the first matmuls wait on ~1.5MB,
                # not the full multi-MB group loads (ci 0..3 for y, ci 0 for m)
                NFC = 4  # group 0's first 4 ci blocks are contiguous columns
                # ci=0 in its own tiny tile (256KB) so the first matmul fires
                # as early as possible; ci 1..3 in a second tile
                y_c0 = sb1.tile([P, 2 * B_SH], bf16)
                nc.scalar.dma_start(out=y_c0[:], in_=yT_pack[0:P, 0:2 * B_SH])
                y_ci0 = sb1.tile([P, (NFC - 1) * 2 * B_SH], bf16)
                nc.scalar.dma_start(out=y_ci0[:],
                                    in_=yT_pack[0:P, 2 * B_SH:NFC * 2 * B_SH])
                m_ci0 = sb1.tile([P, 2 * 512], bf16)
                nc.sync.dma_start(out=m_ci0[:], in_=mT_pack[0:P, 0:2 * 512])

                # resident packed yT groups: [128, 8ci * (hi|lo) * 512b]
                ycols = (NCI // YG) * 2 * B_SH
                yt = []
                for g in range(YG):
                    t = sb1.tile([P, ycols], bf16, name=f"yt{g}", tag=f"yt{g}")
                    nc.scalar.dma_start(out=t[:], in_=yT_pack[g * P:(g + 1) * P, :])
                    yt.append(t)

                def y_slice(ci, s, j):
                    if ci == 0:
                        return y_c0[:, s * B_SH + j * P:s * B_SH + (j + 1) * P]
                    if ci < NFC:
                        off = (ci - 1) * (2 * B_SH) + s * B_SH + j * P
                        return y_ci0[:, off:off + P]
                    g, cl = divmod(ci, NCI // YG)
                    off = cl * (2 * B_SH) + s * B_SH + j * P
                    return yt[g][:, off:off + P]

                F_sb = [sb1.tile([P, N_CLUST], f32, name=f"F{j}", tag=f"F{j}")
                        for j in range(NJ)]
                val = [sb1.tile([P, N_CLUST], f32, name=f"val{j}", tag=f"val{j}")
                       for j in range(NJ)]
                w_enc = sb1.tile([P, KCHUNKS], f32)
                rmax_h = sb1.tile([P, 2 * NJ], f32)  # per-half row maxes
                rmax = sb1.tile([P, NJ], f32)

                mcols = MG_CI * 2 * 512
                p3_loaded = False
                for h in range(2):  # cluster halves (512 wide, one psum bank)
                    ks = slice(h * 512, (h + 1) * 512)
                    psum_js = [ps8.tile([P, 512], f32, name=f"ps{h}{j}", tag="ps",
                                        space="PSUM") for j in range(NJ)]
                    for gq in range(MG):
                        mt = mpool.tile([P, mcols], bf16, name=f"mt{h}{gq}", tag="mt")
                        row0 = (h * MG + gq) * P
                        nc.sync.dma_start(out=mt[:], in_=mT_pack[row0:row0 + P, :])
                        if h == 0 and gq == MG - 1 and not p3_loaded:
                            # queue phase-3 loads behind the first half's stream
                            nc.sync.dma_start(out=m_sb[:], in_=m_nat[:])
                            nc.sync.dma_start(out=sd_sb[:], in_=sd_nat[:])
                            p3_loaded = True
                        for cl in range(MG_CI):
                            ci = gq * MG_CI + cl
                            if h == 0 and ci == 0:
                                mh = m_ci0[:, 0:512]
                                ml = m_ci0[:, 512:1024]
                            else:
                                mh = mt[:, cl * 1024:cl * 1024 + 512]
                                ml = mt[:, cl * 1024 + 512:cl * 1024 + 1024]
                            first = ci == 0
                            last = ci == NCI - 1
                            for j in range(NJ):
                                pj = psum_js[j]
                                nc.tensor.matmul(out=pj[:], lhsT=y_slice(ci, 0, j),
                                                 rhs=mh, start=first, stop=False)
                                nc.tensor.matmul(out=pj[:], lhsT=y_slice(ci, 0, j),
                                                 rhs=ml, start=False, stop=False)
                                nc.tensor.matmul(out=pj[:], lhsT=y_slice(ci, 1, j),
                                                 rhs=mh, start=False, stop=last)
                    # F = psum + (-0.5*||m||^2); partial row-max per half
                    for j in range(NJ):
                        nc.vector.tensor_tensor(out=F_sb[j][:, ks], in0=psum_js[j][:],
                                                in1=msq_t[:, ks], op=AO.add)
                        nc.vector.tensor_reduce(out=rmax_h[:, h * NJ + j:h * NJ + j + 1],
                                                in_=F_sb[j][:, ks], axis=AX.X,
                                                op=AO.max)

                # final row max + equality mask -> winner encodings (b_global+1)
                for j in range(NJ):
                    nc.vector.tensor_tensor(out=rmax[:, j:j + 1],
                                            in0=rmax_h[:, j:j + 1],
                                            in1=rmax_h[:, NJ + j:NJ + j + 1],
                                            op=AO.max)
                    nc.vector.tensor_scalar(out=val[j][:], in0=F_sb[j][:],
                                            scalar1=rmax[:, j:j + 1],
                                            scalar2=bg_t[:, j:j + 1],
                                            op0=AO.is_equal, op1=AO.mult)

                # combine batch subtiles elementwise first (valid: partial maxes
                # over disjoint b subsets), then one transpose per cluster chunk
                vmax = sb1.tile([P, N_CLUST], f32)
                nc.vector.tensor_tensor(out=vmax[:], in0=val[0][:], in1=val[1][:],
                                        op=AO.max)
                nc.vector.tensor_tensor(out=vmax[:], in0=vmax[:], in1=val[2][:],
                                        op=AO.max)
                nc.vector.tensor_tensor(out=vmax[:], in0=vmax[:], in1=val[3][:],
                                        op=AO.max)
                for kc in range(KCHUNKS):
                    pT = ps8.tile([P, P], f32, name=f"pT{kc}", tag="ps",
                                  space="PSUM")
                    nc.tensor.transpose(out=pT[:],
                                        in_=vmax[:, kc * P:(kc + 1) * P],
                                        identity=ident[:])
                    nc.vector.tensor_reduce(out=w_enc[:, kc:kc + 1], in_=pT[:],
                                            axis=AX.X, op=AO.max)
                # transpose w_enc -> [kc, p] rows, single contiguous DMA out
                pWT = ps8.tile([KCHUNKS, P], f32, name="pWT", tag="ps", space="PSUM")
                nc.tensor.transpose(out=pWT[:], in_=w_enc[:], identity=ident[:])
                w_encT = sb1.tile([KCHUNKS, P], f32)
                nc.vector.tensor_copy(out=w_encT[:], in_=pWT[:])
                nc.scalar.dma_start(out=cc_in[:], in_=w_encT[:])

                # ------- Phase 2: ReduceScatter(max) of winner encodings -------
                nc.gpsimd.collective_compute(
                    "ReduceScatter", AO.max, replica_groups=[core_ids],
                    ins=[cc_in[:]], outs=[rs_out[:]])

            # ---------------- Phase 3: gather + EMA update ----------------
            with tc.tile_pool(name="p3sbuf", bufs=1) as sbp, \
                 tc.tile_pool(name="p3psum", bufs=1, space="PSUM") as psp:
                rs_sb = sbp.tile([1, P], f32)
                nc.scalar.dma_start(out=rs_sb[:], in_=rs_out[:])
                pW = psp.tile([P, 1], f32, space="PSUM")
                nc.tensor.transpose(out=pW[:], in_=rs_sb[:],
                                    identity=ident[0:1, 0:1])
                w_own = sbp.tile([P, 1], f32)
                nc.vector.tensor_copy(out=w_own[:], in_=pW[:])

                gidx_f = sbp.tile([P, 1], f32)
                nc.vector.tensor_scalar(out=gidx_f[:], in0=w_own[:], scalar1=-1.0,
                                        scalar2=0.0, op0=AO.add, op1=AO.max)
                gidx_i = sbp.tile([P, 1], mybir.dt.int32)
                nc.vector.tensor_copy(out=gidx_i[:], in_=gidx_f[:])
                nbm = sbp.tile([P, 1], f32)
                nc.vector.tensor_scalar(out=nbm[:], in0=w_own[:], scalar1=0.5,
                                        scalar2=-0.99, op0=AO.is_gt, op1=AO.mult)
                ssd = sbp.tile([P, 1], f32)
                nc.vector.tensor_scalar(out=ssd[:], in0=w_own[:], scalar1=0.5,
                                        scalar2=1e-3, op0=AO.is_gt, op1=AO.mult)
                # csd = 1 - 0.01*a = 1 + nbm*(1/99); 1/99 rounding only
                # perturbs sd's 0.99 factor at the 1e-8 level
                csd = sbp.tile([P, 1], f32)
                nc.vector.tensor_scalar(out=csd[:], in0=nbm[:],
                                        scalar1=float(np.float32(1.0 / 99.0)),
                                        scalar2=1.0, op0=AO.mult, op1=AO.add)

                NQW = CL // NQ
                yg_quarters = []
                for q in range(NQ):
                    ygq = sbp.tile([K_SH, NQW], f32, name=f"yg{q}")
                    nc.gpsimd.indirect_dma_start(
                        out=ygq[:], out_offset=None, in_=y_q[q][:],
                        in_offset=bass.IndirectOffsetOnAxis(ap=gidx_i[:, 0:1], axis=0))
                    yg_quarters.append(ygq)

                # column-split pipeline so ACT/DVE/DMA overlap
                diff = sbp.tile([K_SH, CL], f32)
                new_m = sbp.tile([K_SH, CL], f32)
                sq = sbp.tile([K_SH, CL], f32)
                new_sd = sbp.tile([K_SH, CL], f32)
                for q in range(NQ):
                    cs = slice(q * NQW, (q + 1) * NQW)
                    nc.vector.tensor_tensor(out=diff[:, cs], in0=m_sb[:, cs],
                                            in1=yg_quarters[q][:], op=AO.subtract)
                    nc.scalar.activation(out=sq[:, cs], in_=diff[:, cs],
                                         func=mybir.ActivationFunctionType.Square,
                                         scale=ssd[:, 0:1])
                    nc.vector.scalar_tensor_tensor(out=new_m[:, cs], in0=diff[:, cs],
                                                   scalar=nbm[:, 0:1], in1=m_sb[:, cs],
                                                   op0=AO.mult, op1=AO.add)
                    nc.vector.scalar_tensor_tensor(out=new_sd[:, cs], in0=sd_sb[:, cs],
                                                   scalar=csd[:, 0:1], in1=sq[:, cs],
                                                   op0=AO.mult, op1=AO.add)
                    nc.sync.dma_start(out=out[0:K_SH, cs], in_=new_m[:, cs])
                    nc.sync.dma_start(out=out[K_SH:2 * K_SH, cs], in_=new_sd[:, cs])

    nc.compile()
    return nc


def _prep_inputs(y, m, sd):
    yf = np.ascontiguousarray(y.reshape(B, CL), dtype=np.float32)
    mf = np.ascontiguousarray(m.reshape(N_CLUST, CL), dtype=np.float32)
    sdf = np.ascontiguousarray(sd.reshape(N_CLUST, CL), dtype=np.float32)

    yT = np.ascontiguousarray(yf.T)          # [CL, B]
    yT_hi = yT.astype(BF16)
    yT_lo = (yT - yT_hi.astype(np.float32)).astype(BF16)

    mT = np.ascontiguousarray(mf.T)          # [CL, N_CLUST]
    mT_hi = mT.astype(BF16)
    mT_lo = (mT - mT_hi.astype(np.float32)).astype(BF16)
    # mT pack: [2h x MG groups] rows of P, cols [MG_CI ci][2 s][512 k]
    # view [NCI, P, N_CLUST] -> chunk ci covers partitions ci*P..
    mh_c = mT_hi.reshape(NCI, P, 2, 512)     # [ci, p, h, 512]
    ml_c = mT_lo.reshape(NCI, P, 2, 512)
    mpk = np.empty((2, MG, P, MG_CI, 2, 512), dtype=BF16)
    for h in range(2):
        for gq in range(MG):
            for cl in range(MG_CI):
                ci = gq * MG_CI + cl
                mpk[h, gq, :, cl, 0, :] = mh_c[ci, :, h, :]
                mpk[h, gq, :, cl, 1, :] = ml_c[ci, :, h, :]
    mT_pack = np.ascontiguousarray(mpk.reshape(2 * MG * P, MG_CI * 2 * 512))

    msq = (mf.astype(np.float64) ** 2).sum(1)
    msqneg = np.ascontiguousarray(
        np.broadcast_to((-0.5 * msq).astype(np.float32), (P, N_CLUST)))

    ident = np.eye(P, dtype=np.float32)
    iota = np.arange(P, dtype=np.float32)
    NQ = 2
    y_quarters = [np.ascontiguousarray(yf[:, q * (CL // NQ):(q + 1) * (CL // NQ)])
                  for q in range(NQ)]

    yh_c = yT_hi.reshape(NCI, P, B)          # [ci, p, b_global]
    yl_c = yT_lo.reshape(NCI, P, B)
    CIG = NCI // YG

    in_maps = []
    for i in range(N_CORES):
        bs = slice(i * B_SH, (i + 1) * B_SH)
        ypk = np.empty((YG, P, CIG, 2, B_SH), dtype=BF16)
        for g in range(YG):
            for cl in range(CIG):
                ci = g * CIG + cl
                ypk[g, :, cl, 0, :] = yh_c[ci, :, bs]
                ypk[g, :, cl, 1, :] = yl_c[ci, :, bs]
        bg = np.empty((P, NJ), np.float32)
        for j in range(NJ):
            bg[:, j] = i * B_SH + j * P + iota + 1.0
        in_maps.append({
            "yT_pack": np.ascontiguousarray(ypk.reshape(YG * P, CIG * 2 * B_SH)),
            "mT_pack": mT_pack,
            "msqneg": msqneg,
            "m_nat": np.ascontiguousarray(mf[i * K_SH:(i + 1) * K_SH]),
            "sd_nat": np.ascontiguousarray(sdf[i * K_SH:(i + 1) * K_SH]),
            **{f"y_q{q}": y_quarters[q] for q in range(NQ)},
            "bglob": bg,
            "ident": ident,
        })
    return in_maps


def _run(inputs, trace=False):
    if "nc" not in _CACHE:
        _CACHE["nc"] = _build()
    nc = _CACHE["nc"]
    in_maps = _prep_inputs(np.asarray(inputs["y"]), np.asarray(inputs["m"]),
                           np.asarray(inputs["sd"]))
    res = run_bass_kernel_spmd(nc, in_maps, list(range(N_CORES)), trace=trace)
    out_full = np.empty((2 * N_CLUST, CL), np.float32)
    for i in range(N_CORES):
        o = res.results[i]["out"]
        out_full[i * K_SH:(i + 1) * K_SH] = o[:K_SH]
        out_full[N_CLUST + i * K_SH:N_CLUST + (i + 1) * K_SH] = o[K_SH:]
    return out_full.reshape(2 * N_CLUST, C, L), res


def kernel(**inputs):
    out, _ = _run(inputs, trace=False)
    return out

